# revision 9
# baseline (speedup 1.0000x reference)
"""BoxFilter (9x9 mean filter, reflect padding) Trainium2 Bass kernel.

Input x: [8, 3, 2048, 2048] f32, r=4. Output same shape.

Strategy (per NeuronCore; pure data parallel, batch b -> core b):
  - H-axis box sum via TensorEngine: banded-matrix matmuls (float32r) with
    reflection folded into edge band matrices and the 1/81 scale folded into
    the coefficients. Per 128-row output tile, accumulate contributions from
    the previous tile's last 4 rows, the current tile, and the next tile's
    first 4 rows into PSUM.
  - W-axis box sum via VectorEngine: reflect-pad the H-summed tile into a
    [128, 2057] SBUF tile (ScalarEngine copies from PSUM), run a prefix-sum
    along the free dim (tensor_tensor_scan), then one shifted subtract:
    out[w] = C[w+9] - C[w].
"""

import os
import sys

import numpy as np

for _p in ("/opt/trn_rl_repo", "/opt/pypackages"):
    if os.path.isdir(_p) and _p not in sys.path:
        sys.path.append(_p)

from contextlib import ExitStack

import concourse.bacc as bacc
import concourse.mybir as mybir
from concourse.tile import TileContext
from concourse.bass_utils import run_bass_kernel_spmd

R = 4
NORM = (2 * R + 1) ** 2  # 81
H = W = 2048
P = 128
NT = H // P  # 16 row tiles per image
NIMG = 3  # images per core (batch b -> core b, 3 channels each)
NCORES = 8
CH = 512  # psum chunk (one bank of f32)
F32 = mybir.dt.float32
F32R = mybir.dt.float32r
PW = W + 2 * R + 1  # padded width 2057


def _band_blocks():
    """Banded H-axis operator blocks for the shifted-tile scheme.

    Input tile t holds image rows [128t-4, 128t+124); tile 0 has rows
    [0,124) in partitions [4,128) (partitions 0..3 zeroed); a tail tile
    holds rows [2044,2048). Out tile t = a_t.T @ tile_t + b_t.T @ halo.
    """
    hop = np.zeros((H, H), np.float32)
    inv = np.float32(1.0) / np.float32(NORM)
    for i in range(H):
        for dh in range(-R, R + 1):
            g = i + dh
            if g < 0:
                g = -g
            elif g > H - 1:
                g = 2 * (H - 1) - g
            hop[i, g] += inv
    a_first = np.zeros((P, P), np.float32)
    a_first[R:, :] = hop[0:P, 0 : P - R].T
    a_mid = np.ascontiguousarray(hop[P : 2 * P, P - R : 2 * P - R].T)
    a_last = np.ascontiguousarray(hop[(NT - 1) * P :, (NT - 1) * P - R : NT * P - R].T)
    b_mid = np.ascontiguousarray(hop[0:P, P - R : P + R].T)  # [8, 128]
    b_last = np.ascontiguousarray(hop[(NT - 1) * P :, NT * P - R :].T)  # [4, 128]
    return a_first, a_mid, a_last, b_mid, b_last


def _build_nc():
    nc = bacc.Bacc("TRN2", target_bir_lowering=False, debug=False, num_devices=1)
    x = nc.declare_dram_parameter("x", [NIMG, H, W], F32R, isOutput=False)
    a_first = nc.declare_dram_parameter("a_first", [P, P], F32R, isOutput=False)
    a_mid = nc.declare_dram_parameter("a_mid", [P, P], F32R, isOutput=False)
    a_last = nc.declare_dram_parameter("a_last", [P, P], F32R, isOutput=False)
    b_mid = nc.declare_dram_parameter("b_mid", [2 * R, P], F32R, isOutput=False)
    b_last = nc.declare_dram_parameter("b_last", [R, P], F32R, isOutput=False)
    y = nc.declare_dram_parameter("y", [NIMG, H, W], F32, isOutput=True)

    with TileContext(nc) as tc, ExitStack() as ctx:
        cpool = ctx.enter_context(tc.tile_pool(name="const", bufs=1))
        c_af = cpool.tile([P, P], F32R, tag="caf")
        c_am = cpool.tile([P, P], F32R, tag="cam")
        c_al = cpool.tile([P, P], F32R, tag="cal")
        c_bm = cpool.tile([2 * R, P], F32R, tag="cbm")
        c_bl = cpool.tile([R, P], F32R, tag="cbl")
        nc.sync.dma_start(out=c_af[:], in_=a_first[:])
        nc.sync.dma_start(out=c_am[:], in_=a_mid[:])
        nc.sync.dma_start(out=c_al[:], in_=a_last[:])
        nc.sync.dma_start(out=c_bm[:], in_=b_mid[:])
        nc.sync.dma_start(out=c_bl[:], in_=b_last[:])

        xin = ctx.enter_context(tc.tile_pool(name="xin", bufs=5))
        xtail = ctx.enter_context(tc.tile_pool(name="xtail", bufs=2))
        psum = ctx.enter_context(tc.tile_pool(name="ps", bufs=2, space="PSUM"))
        padp = ctx.enter_context(tc.tile_pool(name="pad", bufs=2))
        czp = ctx.enter_context(tc.tile_pool(name="cz", bufs=2))
        outp = ctx.enter_context(tc.tile_pool(name="out", bufs=3))

        for img in range(NIMG):
            tiles = {}
            tiles[0] = xin.tile([P, W], F32R, tag="xin", name=f"xin_{img}_0")
            nc.vector.memset(tiles[0][0:R, :].bitcast(F32), 0.0)
            nc.sync.dma_start(out=tiles[0][R:P, :], in_=x[img, 0 : P - R, :])
            for t in range(NT):
                if t + 1 < NT:
                    tiles[t + 1] = xin.tile(
                        [P, W], F32R, tag="xin", name=f"xin_{img}_{t+1}"
                    )
                    nc.sync.dma_start(
                        out=tiles[t + 1][:],
                        in_=x[img, (t + 1) * P - R : (t + 2) * P - R, :],
                    )
                elif t + 1 == NT:
                    tiles[NT] = xtail.tile(
                        [R, W], F32R, tag="xtail", name=f"xtail_{img}"
                    )
                    nc.sync.dma_start(out=tiles[NT][:], in_=x[img, H - R :, :])
                s = psum.tile([P, W], F32, tag="ps")
                c_a = c_af if t == 0 else (c_al if t == NT - 1 else c_am)
                c_b = c_bl if t == NT - 1 else c_bm
                kb = R if t == NT - 1 else 2 * R
                for c in range(W // CH):
                    sl = slice(c * CH, (c + 1) * CH)
                    nc.tensor.matmul(
                        s[:, sl],
                        c_a[:],
                        tiles[t][:, sl],
                        start=True,
                        stop=False,
                    )
                    nc.tensor.matmul(
                        s[:, sl],
                        c_b[:],
                        tiles[t + 1][0:kb, sl],
                        start=False,
                        stop=True,
                    )
                pt = padp.tile([P, PW], F32, tag="pad")
                nc.gpsimd.memset(pt[:, 0:1], 0.0)
                nc.scalar.copy(pt[:, R + 1 : R + 1 + W], s[:, :])
                # reflect pads (reversed order via negative-stride APs)
                nc.scalar.copy(pt[:, 1 : R + 1], s[:, R:0:-1])
                nc.scalar.copy(pt[:, R + 1 + W :], s[:, W - 2 : W - 2 - R : -1])
                cz = czp.tile([P, PW], F32, tag="cz")
                nc.vector.tensor_tensor_scan(
                    out=cz[:],
                    data0=pt[:],
                    data1=cz[:],
                    initial=0.0,
                    op0=mybir.AluOpType.add,
                    op1=mybir.AluOpType.bypass,
                )
                o = outp.tile([P, W], F32, tag="out")
                nc.vector.tensor_sub(o[:], cz[:, 2 * R + 1 :], cz[:, 0:W])
                nc.scalar.dma_start(out=y[img, t * P : (t + 1) * P, :], in_=o[:])
    nc.finalize()
    return nc


_CACHE = {}


def _get_setup():
    if "nc" not in _CACHE:
        _CACHE["nc"] = _build_nc()
        _CACHE["blocks"] = _band_blocks()
    return _CACHE["nc"], _CACHE["blocks"]


def kernel(x, r):
    r = int(np.asarray(r))
    assert r == R, f"kernel hardcoded for r={R}, got {r}"
    x = np.asarray(x)
    assert x.shape == (8, 3, H, W) and x.dtype == np.float32, (x.shape, x.dtype)

    nc, (a_first, a_mid, a_last, b_mid, b_last) = _get_setup()
    consts = {
        "a_first": a_first,
        "a_mid": a_mid,
        "a_last": a_last,
        "b_mid": b_mid,
        "b_last": b_last,
    }
    in_maps = [
        {"x": np.ascontiguousarray(x[core]), **consts} for core in range(NCORES)
    ]
    res = run_bass_kernel_spmd(nc, in_maps, core_ids=list(range(NCORES)))
    out = np.stack([res.results[i]["y"] for i in range(NCORES)], axis=0)
    return out.reshape(8, 3, H, W)


def _in_maps(x):
    """in_maps for run_bass_kernel_spmd (used by the test harness)."""
    _, (a_first, a_mid, a_last, b_mid, b_last) = _get_setup()
    consts = {
        "a_first": a_first,
        "a_mid": a_mid,
        "a_last": a_last,
        "b_mid": b_mid,
        "b_last": b_last,
    }
    return [
        {"x": np.ascontiguousarray(x[core]), **consts} for core in range(NCORES)
    ]


if __name__ == "__main__":
    rng = np.random.default_rng(0)
    x = rng.standard_normal((8, 3, H, W), dtype=np.float32)
    y = kernel(x, 4)
    print("ran:", y.shape, y.dtype)


# revision 12
# speedup vs baseline: 1.0573x; 1.0573x over previous
"""BoxFilter (9x9 mean filter, reflect padding) Trainium2 Bass kernel.

Input x: [8, 3, 2048, 2048] f32, r=4. Output same shape.

Strategy (per NeuronCore; pure data parallel, batch b -> core b):
  - H-axis box sum via TensorEngine: banded-matrix matmuls (float32r) with
    reflection folded into edge band matrices and the 1/81 scale folded into
    the coefficients. Per 128-row output tile, accumulate contributions from
    the previous tile's last 4 rows, the current tile, and the next tile's
    first 4 rows into PSUM.
  - W-axis box sum via VectorEngine: reflect-pad the H-summed tile into a
    [128, 2057] SBUF tile (ScalarEngine copies from PSUM), run a prefix-sum
    along the free dim (tensor_tensor_scan), then one shifted subtract:
    out[w] = C[w+9] - C[w].
"""

import os
import sys

import numpy as np

for _p in ("/opt/trn_rl_repo", "/opt/pypackages"):
    if os.path.isdir(_p) and _p not in sys.path:
        sys.path.append(_p)

from contextlib import ExitStack

import concourse.bacc as bacc
import concourse.mybir as mybir
from concourse.tile import TileContext
from concourse.bass_utils import run_bass_kernel_spmd

R = 4
NORM = (2 * R + 1) ** 2  # 81
H = W = 2048
P = 128
NT = H // P  # 16 row tiles per image
NIMG = 3  # images per core (batch b -> core b, 3 channels each)
NCORES = 8
CH = 512  # psum chunk (one bank of f32)
F32 = mybir.dt.float32
F32R = mybir.dt.float32r
PW = W + 2 * R + 1  # padded width 2057


M_EDGE = P - R  # 124-row edge tiles
M_MID = P - 2 * R  # 120-row interior tiles
SIZES = [M_EDGE] + [M_MID] * 15 + [M_EDGE]  # 17 output tiles per image
OFFS = [0]
for _m in SIZES:
    OFFS.append(OFFS[-1] + _m)
assert OFFS[-1] == H
NTILE = len(SIZES)


def _band_blocks():
    """Banded H-axis operator blocks (lhsT, [K=128, M]) for the 17-tile scheme.

    Output tile t covers rows [OFFS[t], OFFS[t]+SIZES[t]); its input tile is
    the 128 rows [120t, 120t+128). Reflection at the image edges is folded
    into a_first / a_last; one K=128 matmul per output tile per psum chunk.
    """
    hop = np.zeros((H, H), np.float32)
    inv = np.float32(1.0) / np.float32(NORM)
    for i in range(H):
        for dh in range(-R, R + 1):
            g = i + dh
            if g < 0:
                g = -g
            elif g > H - 1:
                g = 2 * (H - 1) - g
            hop[i, g] += inv
    a_first = np.ascontiguousarray(hop[0:M_EDGE, 0:P].T)
    a_mid = np.ascontiguousarray(hop[M_EDGE : M_EDGE + M_MID, M_MID : M_MID + P].T)
    a_last = np.ascontiguousarray(hop[H - M_EDGE :, H - P :].T)
    return a_first, a_mid, a_last


def _build_nc():
    nc = bacc.Bacc("TRN2", target_bir_lowering=False, debug=False, num_devices=1)
    x = nc.declare_dram_parameter("x", [NIMG, H, W], F32R, isOutput=False)
    a_first = nc.declare_dram_parameter("a_first", [P, M_EDGE], F32R, isOutput=False)
    a_mid = nc.declare_dram_parameter("a_mid", [P, M_MID], F32R, isOutput=False)
    a_last = nc.declare_dram_parameter("a_last", [P, M_EDGE], F32R, isOutput=False)
    y = nc.declare_dram_parameter("y", [NIMG, H, W], F32, isOutput=True)

    with TileContext(nc) as tc, ExitStack() as ctx:
        cpool = ctx.enter_context(tc.tile_pool(name="const", bufs=1))
        c_af = cpool.tile([P, M_EDGE], F32R, tag="caf")
        c_am = cpool.tile([P, M_MID], F32R, tag="cam")
        c_al = cpool.tile([P, M_EDGE], F32R, tag="cal")
        nc.sync.dma_start(out=c_af[:], in_=a_first[:])
        nc.sync.dma_start(out=c_am[:], in_=a_mid[:])
        nc.sync.dma_start(out=c_al[:], in_=a_last[:])

        xin = ctx.enter_context(tc.tile_pool(name="xin", bufs=8))
        psum = ctx.enter_context(tc.tile_pool(name="ps", bufs=2, space="PSUM"))
        padp = ctx.enter_context(tc.tile_pool(name="pad", bufs=3))
        rsp = ctx.enter_context(tc.tile_pool(name="rs", bufs=4))
        outp = ctx.enter_context(tc.tile_pool(name="out", bufs=4))

        for img in range(NIMG):
            tiles = {}
            for t in range(NTILE):
                if t not in tiles:
                    tiles[t] = xin.tile([P, W], F32R, tag="xin", name=f"xin_{img}_{t}")
                    nc.sync.dma_start(
                        out=tiles[t][:], in_=x[img, M_MID * t : M_MID * t + P, :]
                    )
                if t + 1 < NTILE:
                    tiles[t + 1] = xin.tile(
                        [P, W], F32R, tag="xin", name=f"xin_{img}_{t+1}"
                    )
                    nc.sync.dma_start(
                        out=tiles[t + 1][:],
                        in_=x[img, M_MID * (t + 1) : M_MID * (t + 1) + P, :],
                    )
                m = SIZES[t]
                o_lo = OFFS[t]
                s = psum.tile([P, W], F32, tag="ps")
                c_a = c_af if t == 0 else (c_al if t == NTILE - 1 else c_am)
                for c in range(W // CH):
                    sl = slice(c * CH, (c + 1) * CH)
                    nc.tensor.matmul(
                        s[0:m, sl],
                        c_a[:, 0:m],
                        tiles[t][:, sl],
                        start=True,
                        stop=True,
                    )
                pt = padp.tile([P, PW], F32, tag="pad")
                nc.gpsimd.memset(pt[0:m, 0:1], 0.0)
                nc.scalar.copy(pt[0:m, R + 1 : R + 1 + W], s[0:m, :])
                # reflect pads (reversed order via negative-stride APs)
                nc.scalar.copy(pt[0:m, 1 : R + 1], s[0:m, R:0:-1])
                nc.scalar.copy(pt[0:m, R + 1 + W :], s[0:m, W - 2 : W - 2 - R : -1])
                # running-box-sum recurrence: out[w] = out[w-1] + P[w+9] - P[w],
                # seeded with sum(P[1..8]).
                rs = rsp.tile([P, 1], F32, tag="rs")
                nc.vector.reduce_sum(
                    out=rs[0:m, :], in_=pt[0:m, 1 : 2 * R + 1], axis=mybir.AxisListType.X
                )
                o = outp.tile([P, W], F32, tag="out")
                nc.vector.tensor_tensor_scan(
                    out=o[0:m, :],
                    data0=pt[0:m, 2 * R + 1 :],
                    data1=pt[0:m, 0:W],
                    initial=rs[0:m, :],
                    op0=mybir.AluOpType.add,
                    op1=mybir.AluOpType.subtract,
                )
                nc.scalar.dma_start(out=y[img, o_lo : o_lo + m, :], in_=o[0:m, :])
    nc.finalize()
    return nc


_CACHE = {}


def _get_setup():
    if "nc" not in _CACHE:
        _CACHE["nc"] = _build_nc()
        _CACHE["blocks"] = _band_blocks()
    return _CACHE["nc"], _CACHE["blocks"]


def kernel(x, r):
    r = int(np.asarray(r))
    assert r == R, f"kernel hardcoded for r={R}, got {r}"
    x = np.asarray(x)
    assert x.shape == (8, 3, H, W) and x.dtype == np.float32, (x.shape, x.dtype)

    nc, (a_first, a_mid, a_last) = _get_setup()
    consts = {"a_first": a_first, "a_mid": a_mid, "a_last": a_last}
    in_maps = [
        {"x": np.ascontiguousarray(x[core]), **consts} for core in range(NCORES)
    ]
    res = run_bass_kernel_spmd(nc, in_maps, core_ids=list(range(NCORES)))
    out = np.stack([res.results[i]["y"] for i in range(NCORES)], axis=0)
    return out.reshape(8, 3, H, W)


def _in_maps(x):
    """in_maps for run_bass_kernel_spmd (used by the test harness)."""
    _, (a_first, a_mid, a_last) = _get_setup()
    consts = {"a_first": a_first, "a_mid": a_mid, "a_last": a_last}
    return [
        {"x": np.ascontiguousarray(x[core]), **consts} for core in range(NCORES)
    ]


if __name__ == "__main__":
    rng = np.random.default_rng(0)
    x = rng.standard_normal((8, 3, H, W), dtype=np.float32)
    y = kernel(x, 4)
    print("ran:", y.shape, y.dtype)


# revision 14
# speedup vs baseline: 1.0744x; 1.0162x over previous
"""BoxFilter (9x9 mean filter, reflect padding) Trainium2 Bass kernel.

Input x: [8, 3, 2048, 2048] f32, r=4. Output same shape.

Strategy (per NeuronCore; pure data parallel, batch b -> core b):
  - H-axis box sum via TensorEngine: banded-matrix matmuls (float32r) with
    reflection folded into edge band matrices and the 1/81 scale folded into
    the coefficients. Per 128-row output tile, accumulate contributions from
    the previous tile's last 4 rows, the current tile, and the next tile's
    first 4 rows into PSUM.
  - W-axis box sum via VectorEngine: reflect-pad the H-summed tile into a
    [128, 2057] SBUF tile (ScalarEngine copies from PSUM), run a prefix-sum
    along the free dim (tensor_tensor_scan), then one shifted subtract:
    out[w] = C[w+9] - C[w].
"""

import os
import sys

import numpy as np

for _p in ("/opt/trn_rl_repo", "/opt/pypackages"):
    if os.path.isdir(_p) and _p not in sys.path:
        sys.path.append(_p)

from contextlib import ExitStack

import concourse.bacc as bacc
import concourse.mybir as mybir
from concourse.tile import TileContext
from concourse.bass_utils import run_bass_kernel_spmd

R = 4
NORM = (2 * R + 1) ** 2  # 81
H = W = 2048
P = 128
NT = H // P  # 16 row tiles per image
NIMG = 3  # images per core (batch b -> core b, 3 channels each)
NCORES = 8
CH = 512  # psum chunk (one bank of f32)
F32 = mybir.dt.float32
F32R = mybir.dt.float32r
PW = W + 2 * R + 1  # padded width 2057


M_EDGE = P - R  # 124-row edge tiles
M_MID = P - 2 * R  # 120-row interior tiles
SIZES = [M_EDGE] + [M_MID] * 15 + [M_EDGE]  # 17 output tiles per image
OFFS = [0]
for _m in SIZES:
    OFFS.append(OFFS[-1] + _m)
assert OFFS[-1] == H
NTILE = len(SIZES)


def _band_blocks():
    """Banded H-axis operator blocks (lhsT, [K=128, M]) for the 17-tile scheme.

    Output tile t covers rows [OFFS[t], OFFS[t]+SIZES[t]); its input tile is
    the 128 rows [120t, 120t+128). Reflection at the image edges is folded
    into a_first / a_last; one K=128 matmul per output tile per psum chunk.
    """
    hop = np.zeros((H, H), np.float32)
    inv = np.float32(1.0) / np.float32(NORM)
    for i in range(H):
        for dh in range(-R, R + 1):
            g = i + dh
            if g < 0:
                g = -g
            elif g > H - 1:
                g = 2 * (H - 1) - g
            hop[i, g] += inv
    a_first = np.ascontiguousarray(hop[0:M_EDGE, 0:P].T)
    a_mid = np.ascontiguousarray(hop[M_EDGE : M_EDGE + M_MID, M_MID : M_MID + P].T)
    a_last = np.ascontiguousarray(hop[H - M_EDGE :, H - P :].T)
    return a_first, a_mid, a_last


def _build_nc():
    nc = bacc.Bacc("TRN2", target_bir_lowering=False, debug=False, num_devices=1)
    x = nc.declare_dram_parameter("x", [NIMG, H, W], F32R, isOutput=False)
    a_first = nc.declare_dram_parameter("a_first", [P, M_EDGE], F32R, isOutput=False)
    a_mid = nc.declare_dram_parameter("a_mid", [P, M_MID], F32R, isOutput=False)
    a_last = nc.declare_dram_parameter("a_last", [P, M_EDGE], F32R, isOutput=False)
    y = nc.declare_dram_parameter("y", [NIMG, H, W], F32, isOutput=True)

    with TileContext(nc) as tc, ExitStack() as ctx:
        cpool = ctx.enter_context(tc.tile_pool(name="const", bufs=1))
        c_af = cpool.tile([P, M_EDGE], F32R, tag="caf")
        c_am = cpool.tile([P, M_MID], F32R, tag="cam")
        c_al = cpool.tile([P, M_EDGE], F32R, tag="cal")
        nc.sync.dma_start(out=c_af[:], in_=a_first[:])
        nc.sync.dma_start(out=c_am[:], in_=a_mid[:])
        nc.sync.dma_start(out=c_al[:], in_=a_last[:])

        xin = ctx.enter_context(tc.tile_pool(name="xin", bufs=4))
        xin1 = ctx.enter_context(tc.tile_pool(name="xin1", bufs=2))
        psum = ctx.enter_context(tc.tile_pool(name="ps", bufs=2, space="PSUM"))
        padp = ctx.enter_context(tc.tile_pool(name="pad", bufs=3))
        rsp = ctx.enter_context(tc.tile_pool(name="rs", bufs=4))
        outp = ctx.enter_context(tc.tile_pool(name="outp", bufs=2))
        outs = ctx.enter_context(tc.tile_pool(name="outs", bufs=3))

        def load_pair(img, tp, tiles):
            """One DMA loads input tiles tp and tp+1 (overlapping 128-row reads
            at 120-row stride) into one [P, 2W] buffer."""
            buf = xin.tile([P, 2 * W], F32R, tag="xin", name=f"xin_{img}_{tp}")
            src = (
                x[img, M_MID * tp : M_MID * tp + P, :]
                .unsqueeze(1)
                .broadcast_to([P, 2, W])
                .copy()
            )
            src.ap[1] = [M_MID * W, 2]
            nc.sync.dma_start(
                out=buf[:].rearrange("p (c w) -> p c w", c=2), in_=src
            )
            tiles[tp] = buf[:, 0:W]
            tiles[tp + 1] = buf[:, W : 2 * W]

        for img in range(NIMG):
            tiles = {}
            load_pair(img, 0, tiles)
            load_pair(img, 2, tiles)
            obuf = None
            for t in range(NTILE):
                # prefetch: keep two pair-loads in flight
                if t % 2 == 0 and t + 4 < NTILE - 1:
                    load_pair(img, t + 4, tiles)
                elif t == 14:
                    tiles[16] = xin1.tile(
                        [P, W], F32R, tag="xin1", name=f"xin1_{img}"
                    )
                    nc.sync.dma_start(
                        out=tiles[16][:], in_=x[img, H - P :, :]
                    )
                m = SIZES[t]
                o_lo = OFFS[t]
                s = psum.tile([P, W], F32, tag="ps")
                c_a = c_af if t == 0 else (c_al if t == NTILE - 1 else c_am)
                for c in range(W // CH):
                    sl = slice(c * CH, (c + 1) * CH)
                    nc.tensor.matmul(
                        s[0:m, sl],
                        c_a[:, 0:m],
                        tiles[t][:, sl],
                        start=True,
                        stop=True,
                    )
                pt = padp.tile([P, PW], F32, tag="pad")
                nc.gpsimd.memset(pt[0:m, 0:1], 0.0)
                nc.scalar.copy(pt[0:m, R + 1 : R + 1 + W], s[0:m, :])
                # reflect pads (reversed order via negative-stride APs)
                nc.scalar.copy(pt[0:m, 1 : R + 1], s[0:m, R:0:-1])
                nc.scalar.copy(pt[0:m, R + 1 + W :], s[0:m, W - 2 : W - 2 - R : -1])
                # running-box-sum recurrence: out[w] = out[w-1] + P[w+9] - P[w],
                # seeded with sum(P[1..8]).
                rs = rsp.tile([P, 1], F32, tag="rs")
                nc.vector.reduce_sum(
                    out=rs[0:m, :], in_=pt[0:m, 1 : 2 * R + 1], axis=mybir.AxisListType.X
                )
                # interior tiles (1,2), (3,4), ... (13,14) pair into one
                # [P, 2W] buffer and store with a single 2MB DMA; tiles
                # 0, 15, 16 store singly.
                paired = 1 <= t <= 14
                if paired:
                    if t % 2 == 1:
                        obuf = outp.tile([P, 2 * W], F32, tag="outp")
                    half = (t + 1) % 2  # t odd -> first half, t even -> second
                    o_ap = obuf[0:m, half * W : half * W + W]
                else:
                    o_single = outs.tile([P, W], F32, tag="outs")
                    o_ap = o_single[0:m, :]
                nc.vector.tensor_tensor_scan(
                    out=o_ap,
                    data0=pt[0:m, 2 * R + 1 :],
                    data1=pt[0:m, 0:W],
                    initial=rs[0:m, :],
                    op0=mybir.AluOpType.add,
                    op1=mybir.AluOpType.subtract,
                )
                if paired and t % 2 == 0:
                    dst = (
                        y[img, OFFS[t - 1] : OFFS[t - 1] + M_MID, :]
                        .unsqueeze(1)
                        .broadcast_to([M_MID, 2, W])
                        .copy()
                    )
                    dst.ap[1] = [M_MID * W, 2]
                    nc.gpsimd.dma_start(
                        out=dst,
                        in_=obuf[0:M_MID, :].rearrange("p (c w) -> p c w", c=2),
                    )
                elif not paired:
                    nc.gpsimd.dma_start(
                        out=y[img, o_lo : o_lo + m, :], in_=o_ap
                    )
    nc.finalize()
    return nc


_CACHE = {}


def _get_setup():
    if "nc" not in _CACHE:
        _CACHE["nc"] = _build_nc()
        _CACHE["blocks"] = _band_blocks()
    return _CACHE["nc"], _CACHE["blocks"]


def kernel(x, r):
    r = int(np.asarray(r))
    assert r == R, f"kernel hardcoded for r={R}, got {r}"
    x = np.asarray(x)
    assert x.shape == (8, 3, H, W) and x.dtype == np.float32, (x.shape, x.dtype)

    nc, (a_first, a_mid, a_last) = _get_setup()
    consts = {"a_first": a_first, "a_mid": a_mid, "a_last": a_last}
    in_maps = [
        {"x": np.ascontiguousarray(x[core]), **consts} for core in range(NCORES)
    ]
    res = run_bass_kernel_spmd(nc, in_maps, core_ids=list(range(NCORES)))
    out = np.stack([res.results[i]["y"] for i in range(NCORES)], axis=0)
    return out.reshape(8, 3, H, W)


def _in_maps(x):
    """in_maps for run_bass_kernel_spmd (used by the test harness)."""
    _, (a_first, a_mid, a_last) = _get_setup()
    consts = {"a_first": a_first, "a_mid": a_mid, "a_last": a_last}
    return [
        {"x": np.ascontiguousarray(x[core]), **consts} for core in range(NCORES)
    ]


if __name__ == "__main__":
    rng = np.random.default_rng(0)
    x = rng.standard_normal((8, 3, H, W), dtype=np.float32)
    y = kernel(x, 4)
    print("ran:", y.shape, y.dtype)


# revision 16
# speedup vs baseline: 1.1449x; 1.0656x over previous
"""BoxFilter (9x9 mean filter, reflect padding) Trainium2 Bass kernel.

Input x: [8, 3, 2048, 2048] f32, r=4. Output same shape.

Strategy (per NeuronCore; pure data parallel, batch b -> core b):
  - H-axis box sum via TensorEngine: banded-matrix matmuls (float32r) with
    reflection folded into edge band matrices and the 1/81 scale folded into
    the coefficients. Per 128-row output tile, accumulate contributions from
    the previous tile's last 4 rows, the current tile, and the next tile's
    first 4 rows into PSUM.
  - W-axis box sum via VectorEngine: reflect-pad the H-summed tile into a
    [128, 2057] SBUF tile (ScalarEngine copies from PSUM), run a prefix-sum
    along the free dim (tensor_tensor_scan), then one shifted subtract:
    out[w] = C[w+9] - C[w].
"""

import os
import sys

import numpy as np

for _p in ("/opt/trn_rl_repo", "/opt/pypackages"):
    if os.path.isdir(_p) and _p not in sys.path:
        sys.path.append(_p)

from contextlib import ExitStack

import concourse.bacc as bacc
import concourse.mybir as mybir
from concourse.tile import TileContext
from concourse.bass_utils import run_bass_kernel_spmd

R = 4
NORM = (2 * R + 1) ** 2  # 81
H = W = 2048
P = 128
NT = H // P  # 16 row tiles per image
NIMG = 3  # images per core (batch b -> core b, 3 channels each)
NCORES = 8
CH = 512  # psum chunk (one bank of f32)
F32 = mybir.dt.float32
F32R = mybir.dt.float32r
PW = W + 2 * R + 1  # padded width 2057


M_EDGE = P - R  # 124-row edge tiles
M_MID = P - 2 * R  # 120-row interior tiles
SIZES = [M_EDGE] + [M_MID] * 15 + [M_EDGE]  # 17 output tiles per image
OFFS = [0]
for _m in SIZES:
    OFFS.append(OFFS[-1] + _m)
assert OFFS[-1] == H
NTILE = len(SIZES)


def _band_blocks():
    """Banded H-axis operator blocks (lhsT, [K=128, M]) for the 17-tile scheme.

    Output tile t covers rows [OFFS[t], OFFS[t]+SIZES[t]); its input tile is
    the 128 rows [120t, 120t+128). Reflection at the image edges is folded
    into a_first / a_last; one K=128 matmul per output tile per psum chunk.
    """
    hop = np.zeros((H, H), np.float32)
    inv = np.float32(1.0) / np.float32(NORM)
    for i in range(H):
        for dh in range(-R, R + 1):
            g = i + dh
            if g < 0:
                g = -g
            elif g > H - 1:
                g = 2 * (H - 1) - g
            hop[i, g] += inv
    a_first = np.ascontiguousarray(hop[0:M_EDGE, 0:P].T)
    a_mid = np.ascontiguousarray(hop[M_EDGE : M_EDGE + M_MID, M_MID : M_MID + P].T)
    a_last = np.ascontiguousarray(hop[H - M_EDGE :, H - P :].T)
    return a_first, a_mid, a_last


def _build_nc():
    nc = bacc.Bacc("TRN2", target_bir_lowering=False, debug=False, num_devices=1)
    x = nc.declare_dram_parameter("x", [NIMG, H, W], F32R, isOutput=False)
    a_first = nc.declare_dram_parameter("a_first", [P, M_EDGE], F32R, isOutput=False)
    a_mid = nc.declare_dram_parameter("a_mid", [P, M_MID], F32R, isOutput=False)
    a_last = nc.declare_dram_parameter("a_last", [P, M_EDGE], F32R, isOutput=False)
    y = nc.declare_dram_parameter("y", [NIMG, H, W], F32, isOutput=True)

    with TileContext(nc) as tc, ExitStack() as ctx:
        cpool = ctx.enter_context(tc.tile_pool(name="const", bufs=1))
        c_af = cpool.tile([P, M_EDGE], F32R, tag="caf")
        c_am = cpool.tile([P, M_MID], F32R, tag="cam")
        c_al = cpool.tile([P, M_EDGE], F32R, tag="cal")
        zcol = cpool.tile([P, 1], F32, tag="zcol")
        nc.sync.dma_start(out=c_af[:], in_=a_first[:])
        nc.sync.dma_start(out=c_am[:], in_=a_mid[:])
        nc.sync.dma_start(out=c_al[:], in_=a_last[:])
        nc.vector.memset(zcol[:], 0.0)

        xin = ctx.enter_context(tc.tile_pool(name="xin", bufs=4))
        xin1 = ctx.enter_context(tc.tile_pool(name="xin1", bufs=2))
        psum = ctx.enter_context(tc.tile_pool(name="ps", bufs=2, space="PSUM"))
        padp = ctx.enter_context(tc.tile_pool(name="pad", bufs=3))
        rsp = ctx.enter_context(tc.tile_pool(name="rs", bufs=4))
        outp = ctx.enter_context(tc.tile_pool(name="outp", bufs=3))
        outs = ctx.enter_context(tc.tile_pool(name="outs", bufs=2))

        def load_pair(img, tp, tiles):
            """One DMA loads input tiles tp and tp+1 (overlapping 128-row reads
            at 120-row stride) into one [P, 2W] buffer."""
            buf = xin.tile([P, 2 * W], F32R, tag="xin", name=f"xin_{img}_{tp}")
            src = (
                x[img, M_MID * tp : M_MID * tp + P, :]
                .unsqueeze(1)
                .broadcast_to([P, 2, W])
                .copy()
            )
            src.ap[1] = [M_MID * W, 2]
            eng = nc.sync if (tp // 2) % 2 == 0 else nc.scalar
            eng.dma_start(out=buf[:].rearrange("p (c w) -> p c w", c=2), in_=src)
            tiles[tp] = buf[:, 0:W]
            tiles[tp + 1] = buf[:, W : 2 * W]

        for img in range(NIMG):
            tiles = {}
            load_pair(img, 0, tiles)
            load_pair(img, 2, tiles)
            obuf = None
            for t in range(NTILE):
                # prefetch: keep two pair-loads in flight
                if t % 2 == 0 and t + 4 < NTILE - 1:
                    load_pair(img, t + 4, tiles)
                elif t == 14:
                    tiles[16] = xin1.tile(
                        [P, W], F32R, tag="xin1", name=f"xin1_{img}"
                    )
                    nc.sync.dma_start(
                        out=tiles[16][:], in_=x[img, H - P :, :]
                    )
                m = SIZES[t]
                o_lo = OFFS[t]
                s = psum.tile([P, W], F32, tag="ps")
                c_a = c_af if t == 0 else (c_al if t == NTILE - 1 else c_am)
                for c in range(W // CH):
                    sl = slice(c * CH, (c + 1) * CH)
                    nc.tensor.matmul(
                        s[0:m, sl],
                        c_a[:, 0:m],
                        tiles[t][:, sl],
                        start=True,
                        stop=True,
                    )
                pt = padp.tile([P, PW], F32, tag="pad")
                nc.scalar.copy(pt[0:m, 0:1], zcol[0:m, :])
                nc.scalar.copy(pt[0:m, R + 1 : R + 1 + W], s[0:m, :])
                # reflect pads (reversed order via negative-stride APs)
                nc.scalar.copy(pt[0:m, 1 : R + 1], s[0:m, R:0:-1])
                nc.scalar.copy(pt[0:m, R + 1 + W :], s[0:m, W - 2 : W - 2 - R : -1])
                # running-box-sum recurrence: out[w] = out[w-1] + P[w+9] - P[w],
                # seeded with sum(P[1..8]).
                rs = rsp.tile([P, 1], F32, tag="rs")
                nc.vector.reduce_sum(
                    out=rs[0:m, :], in_=pt[0:m, 1 : 2 * R + 1], axis=mybir.AxisListType.X
                )
                # interior tiles (1,2), (3,4), ... (13,14) pair into one
                # [P, 2W] buffer and store with a single 2MB DMA; tiles
                # 0, 15, 16 store singly.
                paired = 1 <= t <= 14
                if paired:
                    if t % 2 == 1:
                        obuf = outp.tile([P, 2 * W], F32, tag="outp")
                    half = (t + 1) % 2  # t odd -> first half, t even -> second
                    o_ap = obuf[0:m, half * W : half * W + W]
                else:
                    o_single = outs.tile([P, W], F32, tag="outs")
                    o_ap = o_single[0:m, :]
                nc.vector.tensor_tensor_scan(
                    out=o_ap,
                    data0=pt[0:m, 2 * R + 1 :],
                    data1=pt[0:m, 0:W],
                    initial=rs[0:m, :],
                    op0=mybir.AluOpType.add,
                    op1=mybir.AluOpType.subtract,
                )
                if paired and t % 2 == 0:
                    dst = (
                        y[img, OFFS[t - 1] : OFFS[t - 1] + M_MID, :]
                        .unsqueeze(1)
                        .broadcast_to([M_MID, 2, W])
                        .copy()
                    )
                    dst.ap[1] = [M_MID * W, 2]
                    seng = nc.scalar if (t // 2) % 2 == 0 else nc.sync
                    seng.dma_start(
                        out=dst,
                        in_=obuf[0:M_MID, :].rearrange("p (c w) -> p c w", c=2),
                    )
                elif not paired:
                    seng = nc.scalar if t % 2 == 0 else nc.sync
                    seng.dma_start(out=y[img, o_lo : o_lo + m, :], in_=o_ap)
    nc.finalize()
    return nc


_CACHE = {}


def _get_setup():
    if "nc" not in _CACHE:
        _CACHE["nc"] = _build_nc()
        _CACHE["blocks"] = _band_blocks()
    return _CACHE["nc"], _CACHE["blocks"]


def kernel(x, r):
    r = int(np.asarray(r))
    assert r == R, f"kernel hardcoded for r={R}, got {r}"
    x = np.asarray(x)
    assert x.shape == (8, 3, H, W) and x.dtype == np.float32, (x.shape, x.dtype)

    nc, (a_first, a_mid, a_last) = _get_setup()
    consts = {"a_first": a_first, "a_mid": a_mid, "a_last": a_last}
    in_maps = [
        {"x": np.ascontiguousarray(x[core]), **consts} for core in range(NCORES)
    ]
    res = run_bass_kernel_spmd(nc, in_maps, core_ids=list(range(NCORES)))
    out = np.stack([res.results[i]["y"] for i in range(NCORES)], axis=0)
    return out.reshape(8, 3, H, W)


def _in_maps(x):
    """in_maps for run_bass_kernel_spmd (used by the test harness)."""
    _, (a_first, a_mid, a_last) = _get_setup()
    consts = {"a_first": a_first, "a_mid": a_mid, "a_last": a_last}
    return [
        {"x": np.ascontiguousarray(x[core]), **consts} for core in range(NCORES)
    ]


if __name__ == "__main__":
    rng = np.random.default_rng(0)
    x = rng.standard_normal((8, 3, H, W), dtype=np.float32)
    y = kernel(x, 4)
    print("ran:", y.shape, y.dtype)
